# revision 11
# baseline (speedup 1.0000x reference)
"""IrrepsLinear Trainium2 kernel: y = per-irrep-block x @ W / sqrt(mul).

Irreps layout: 256x0e + 128x1o + 64x2e -> blocks of width 256*1, 128*3, 64*5.
Data-parallel over 8 NeuronCores: each core gets 12500 nodes.

Strategy (v7 = v2 champion + packed weights + streamed stores):
  - fp16 DRAM IO halves HBM traffic; fp16 matmuls -> fp32 PSUM -> fp16 evac.
  - Host pre-permutes features so each 128-row K-group is one contiguous
    block per window; 5 uniform windows of 2500 nodes, monolithic window
    loads, bufs=2 (pushing DMA harder trips the HW DMA throttle).
  - ALL weights packed into one [128, 832] DRAM tensor -> a single DMA
    dispatched first on the SP ring (v2 serialized 5 dispatches ahead of
    the first x load; weights on the ACT ring arrive far too late).
  - Each window's ya store is split: columns [0, 1536) go out mid-window
    as soon as their evacs land, the rest at window end. Stores enter the
    DMA stream earlier and the end-of-run store drain shrinks.
  - Block2 m-pairs (m0,m1),(m2,m3) via 128x128 block-diag W2; m4 plain.
  - 8 one-bank PSUM tiles rotate per 512-slice; evac alternates DVE/ACT.
"""

import numpy as np

NCORES = 8
N_TOTAL = 100000
NSH = N_TOTAL // NCORES   # 12500 nodes per core
NW = 5
SW = NSH // NW            # 2500
D = 960
MMW = 512                 # matmul slice width (= one fp32 PSUM bank)
CUT = 3 * MMW             # early-store column cut (1536)

DT_IO = "float16"
_BUILD_CACHE = {}


def _perm():
    p = list(range(256))
    for m in range(3):
        p += [256 + 3 * i + m for i in range(128)]
    for m in range(5):
        p += [640 + 5 * i + m for i in range(64)]
    return np.asarray(p, dtype=np.int64)

_PERM = _perm()


def _build_program():
    import concourse.bass as bass  # noqa: F401
    import concourse.bacc as bacc
    import concourse.mybir as mybir
    import concourse.tile as tile

    key = (DT_IO, MMW, SW, "v7")
    if key in _BUILD_CACHE:
        return _BUILD_CACHE[key]

    dt = getattr(mybir.dt, DT_IO)
    f32 = mybir.dt.float32

    nc = bacc.Bacc(
        "TRN2", target_bir_lowering=False, debug=False, enable_asserts=False
    )
    xa = nc.dram_tensor("xa", [NW, 128, 7 * SW], dt, kind="ExternalInput").ap()
    xbd = nc.dram_tensor("xb", [NW, 64, SW], dt, kind="ExternalInput").ap()
    # packed weights: [0:256) w0 rows 0-127 | [256:512) w0 rows 128-255 |
    # [512:640) w1 | [640:768) w2 block-diag | [768:832) w2 plain
    wp = nc.dram_tensor("wp", [128, 832], dt, kind="ExternalInput").ap()
    ya = nc.dram_tensor("ya", [NW, 128, 7 * SW], dt, kind="ExternalOutput").ap()
    ybd = nc.dram_tensor("yb", [NW, 64, SW], dt, kind="ExternalOutput").ap()

    with tile.TileContext(nc) as tc:
        with (
            tc.tile_pool(name="const", bufs=1) as cpool,
            tc.tile_pool(name="xin", bufs=2) as xpool,
            tc.tile_pool(name="yst", bufs=2) as ypool,
            tc.tile_pool(name="ps", bufs=8, space="PSUM") as pspool,
        ):
            # one weight DMA, first in the SP ring
            wpt = cpool.tile([128, 832], dt, name="wpt", tag="wpt")
            nc.sync.dma_start(wpt[:], wp[:, :])
            w0t0 = wpt[:, 0:256]
            w0t1 = wpt[:, 256:512]
            w1t = wpt[:, 512:640]
            w2dt = wpt[:, 640:768]
            w2st = wpt[0:64, 768:832]

            n_evac = 0

            def evac(dst, src):
                nonlocal n_evac
                n_evac += 1
                if n_evac % 2:
                    nc.vector.tensor_copy(dst, src)
                else:
                    nc.scalar.copy(dst, src)

            slices = [
                (i * MMW, min((i + 1) * MMW, SW))
                for i in range((SW + MMW - 1) // MMW)
            ]

            for w in range(NW):
                xat = xpool.tile([128, 7 * SW], dt, name=f"xa{w}", tag="xa")
                nc.sync.dma_start(xat[:], xa[w])
                xbt = xpool.tile([64, SW], dt, name=f"xb{w}", tag="xb")
                nc.sync.dma_start(xbt[:], xbd[w])
                yat = ypool.tile([128, 7 * SW], dt, name=f"ya{w}", tag="ya")
                ybt = ypool.tile([64, SW], dt, name=f"yb{w}", tag="yb")

                for si, (lo, hi) in enumerate(slices):
                    n = hi - lo

                    def pst(nm):
                        return pspool.tile(
                            [128, MMW], f32, name=f"{nm}_{w}_{lo}", tag="ps"
                        )

                    # block0: 256x0e (K=256 via 2 accum steps, M=256 via 2 obs)
                    for ob in range(2):
                        ps = pst(f"ps_b0_{ob}")
                        oc = slice(128 * ob, 128 * (ob + 1))
                        nc.tensor.matmul(
                            ps[:, :n], w0t0[:, oc], xat[:, 0 * SW + lo : 0 * SW + hi],
                            start=True, stop=False,
                        )
                        nc.tensor.matmul(
                            ps[:, :n], w0t1[:, oc], xat[:, 1 * SW + lo : 1 * SW + hi],
                            start=False, stop=True,
                        )
                        evac(yat[:, ob * SW + lo : ob * SW + hi], ps[:, :n])

                    # block1: 128x1o, 3 m-components
                    for m in range(3):
                        ps = pst(f"ps_b1_{m}")
                        t = 2 + m
                        nc.tensor.matmul(
                            ps[:, :n], w1t[:], xat[:, t * SW + lo : t * SW + hi],
                            start=True, stop=True,
                        )
                        evac(yat[:, t * SW + lo : t * SW + hi], ps[:, :n])

                    # block2: m-pairs via block-diag W2 (full PE width)
                    for g in range(2):
                        ps = pst(f"ps_b2_{g}")
                        t = 5 + g
                        nc.tensor.matmul(
                            ps[:, :n], w2dt[:], xat[:, t * SW + lo : t * SW + hi],
                            start=True, stop=True,
                        )
                        evac(yat[:, t * SW + lo : t * SW + hi], ps[:, :n])

                    # block2 m=4: plain 64-wide matmul
                    ps = pst("ps_b2_4")
                    nc.tensor.matmul(
                        ps[0:64, :n], w2st[:], xbt[:, lo:hi],
                        start=True, stop=True,
                    )
                    evac(ybt[:, lo:hi], ps[0:64, :n])

                    # early piece of the ya store: columns [0, CUT)
                    if si == 2:
                        yv = yat[:].rearrange("p (t n) -> p t n", t=7)
                        dv = ya[w].rearrange("p (t n) -> p t n", t=7)
                        nc.scalar.dma_start(dv[:, :, 0:CUT], yv[:, :, 0:CUT])

                # remainder stores on the ACT ring
                yv = yat[:].rearrange("p (t n) -> p t n", t=7)
                dv = ya[w].rearrange("p (t n) -> p t n", t=7)
                nc.scalar.dma_start(dv[:, :, CUT:SW], yv[:, :, CUT:SW])
                nc.scalar.dma_start(ybd[w], ybt[:])

    nc.compile()
    _BUILD_CACHE[key] = nc
    return nc


TRACE = False
LAST_RESULT = None


def kernel(x, W0, W1, W2):
    from concourse import bass_utils

    nc = _build_program()

    npdt = np.float16 if DT_IO == "float16" else None
    if npdt is None:
        import ml_dtypes
        npdt = ml_dtypes.bfloat16

    w0s = (np.asarray(W0, np.float32) / np.sqrt(256.0)).astype(npdt)
    w1s = (np.asarray(W1, np.float32) / np.sqrt(128.0)).astype(npdt)
    w2 = (np.asarray(W2, np.float32) / np.sqrt(64.0)).astype(npdt)
    wp = np.zeros((128, 832), dtype=npdt)
    wp[:, 0:256] = w0s[0:128]
    wp[:, 256:512] = w0s[128:256]
    wp[:, 512:640] = w1s
    wp[0:64, 640:704] = w2
    wp[64:128, 704:768] = w2
    wp[0:64, 768:832] = w2

    xh = np.asarray(x)[:, _PERM].astype(npdt)
    A = xh.reshape(NCORES, NW, SW, D)
    xa_all = np.ascontiguousarray(
        A[:, :, :, :896].reshape(NCORES, NW, SW, 7, 128).transpose(0, 1, 4, 3, 2)
    ).reshape(NCORES, NW, 128, 7 * SW)
    xb_all = np.ascontiguousarray(A[:, :, :, 896:].transpose(0, 1, 3, 2))

    in_maps = []
    for c in range(NCORES):
        in_maps.append({"xa": xa_all[c], "xb": xb_all[c], "wp": wp})

    res = bass_utils.run_bass_kernel_spmd(
        nc, in_maps, core_ids=list(range(NCORES)), trace=TRACE
    )
    global LAST_RESULT
    LAST_RESULT = res

    out = np.empty((N_TOTAL, D), dtype=np.float32)
    Yp = np.empty((NCORES, NW, SW, D), dtype=np.float32)
    for c in range(NCORES):
        yac = res.results[c]["ya"]    # [NW, 128, 7*SW]
        ybc = res.results[c]["yb"]    # [NW, 64, SW]
        Yp[c, :, :, :896] = (
            yac.reshape(NW, 128, 7, SW).transpose(0, 3, 2, 1).reshape(NW, SW, 896)
        )
        Yp[c, :, :, 896:] = ybc.transpose(0, 2, 1)
    out[:, _PERM] = Yp.reshape(N_TOTAL, D)
    return out


# revision 12
# speedup vs baseline: 1.0746x; 1.0746x over previous
"""IrrepsLinear Trainium2 kernel: y = per-irrep-block x @ W / sqrt(mul).

Irreps layout: 256x0e + 128x1o + 64x2e -> blocks of width 256*1, 128*3, 64*5.
Data-parallel over 8 NeuronCores: each core gets 12500 nodes.

Strategy (v8 = v2 champion + packed single weight DMA):
  - fp16 DRAM IO halves HBM traffic; fp16 matmuls -> fp32 PSUM -> fp16 evac.
  - Host pre-permutes features so each 128-row K-group is one contiguous
    block per window; 5 uniform windows of 2500 nodes, monolithic window
    loads, bufs=2 (pushing DMA harder trips the HW DMA throttle).
  - ALL weights packed into one [128, 832] DRAM tensor -> a single DMA
    dispatched first on the SP ring (v2 serialized 5 dispatches ahead of
    the first x load; weights on the ACT ring arrive far too late).
  - Block2 m-pairs (m0,m1),(m2,m3) via 128x128 block-diag W2; m4 plain.
  - 8 one-bank PSUM tiles rotate per 512-slice; evac alternates DVE/ACT.
"""

import numpy as np

NCORES = 8
N_TOTAL = 100000
NSH = N_TOTAL // NCORES   # 12500 nodes per core
NW = 5
SW = NSH // NW            # 2500
D = 960
MMW = 512                 # matmul slice width (= one fp32 PSUM bank)
CUT = 3 * MMW             # early-store column cut (1536)

DT_IO = "float16"
_BUILD_CACHE = {}


def _perm():
    p = list(range(256))
    for m in range(3):
        p += [256 + 3 * i + m for i in range(128)]
    for m in range(5):
        p += [640 + 5 * i + m for i in range(64)]
    return np.asarray(p, dtype=np.int64)

_PERM = _perm()


def _build_program():
    import concourse.bass as bass  # noqa: F401
    import concourse.bacc as bacc
    import concourse.mybir as mybir
    import concourse.tile as tile

    key = (DT_IO, MMW, SW, "v8a")
    if key in _BUILD_CACHE:
        return _BUILD_CACHE[key]

    dt = getattr(mybir.dt, DT_IO)
    f32 = mybir.dt.float32

    nc = bacc.Bacc(
        "TRN2", target_bir_lowering=False, debug=False, enable_asserts=False
    )
    xa = nc.dram_tensor("xa", [NW, 128, 7 * SW], dt, kind="ExternalInput").ap()
    xbd = nc.dram_tensor("xb", [NW, 64, SW], dt, kind="ExternalInput").ap()
    # packed weights: [0:256) w0 rows 0-127 | [256:512) w0 rows 128-255 |
    # [512:640) w1 | [640:768) w2 block-diag | [768:832) w2 plain
    wp = nc.dram_tensor("wp", [128, 832], dt, kind="ExternalInput").ap()
    ya = nc.dram_tensor("ya", [NW, 128, 7 * SW], dt, kind="ExternalOutput").ap()
    ybd = nc.dram_tensor("yb", [NW, 64, SW], dt, kind="ExternalOutput").ap()

    with tile.TileContext(nc) as tc:
        with (
            tc.tile_pool(name="const", bufs=1) as cpool,
            tc.tile_pool(name="xin", bufs=2) as xpool,
            tc.tile_pool(name="yst", bufs=2) as ypool,
            tc.tile_pool(name="ps", bufs=8, space="PSUM") as pspool,
        ):
            # one weight DMA, first in the SP ring
            wpt = cpool.tile([128, 832], dt, name="wpt", tag="wpt")
            nc.sync.dma_start(wpt[:], wp[:, :])
            w0t0 = wpt[:, 0:256]
            w0t1 = wpt[:, 256:512]
            w1t = wpt[:, 512:640]
            w2dt = wpt[:, 640:768]
            w2st = wpt[0:64, 768:832]

            n_evac = 0

            def evac(dst, src):
                nonlocal n_evac
                n_evac += 1
                if n_evac % 2:
                    nc.vector.tensor_copy(dst, src)
                else:
                    nc.scalar.copy(dst, src)

            slices = [
                (i * MMW, min((i + 1) * MMW, SW))
                for i in range((SW + MMW - 1) // MMW)
            ]

            for w in range(NW):
                xat = xpool.tile([128, 7 * SW], dt, name=f"xa{w}", tag="xa")
                nc.sync.dma_start(xat[:], xa[w])
                xbt = xpool.tile([64, SW], dt, name=f"xb{w}", tag="xb")
                nc.sync.dma_start(xbt[:], xbd[w])
                yat = ypool.tile([128, 7 * SW], dt, name=f"ya{w}", tag="ya")
                ybt = ypool.tile([64, SW], dt, name=f"yb{w}", tag="yb")

                for si, (lo, hi) in enumerate(slices):
                    n = hi - lo

                    def pst(nm):
                        return pspool.tile(
                            [128, MMW], f32, name=f"{nm}_{w}_{lo}", tag="ps"
                        )

                    # block0: 256x0e (K=256 via 2 accum steps, M=256 via 2 obs)
                    for ob in range(2):
                        ps = pst(f"ps_b0_{ob}")
                        oc = slice(128 * ob, 128 * (ob + 1))
                        nc.tensor.matmul(
                            ps[:, :n], w0t0[:, oc], xat[:, 0 * SW + lo : 0 * SW + hi],
                            start=True, stop=False,
                        )
                        nc.tensor.matmul(
                            ps[:, :n], w0t1[:, oc], xat[:, 1 * SW + lo : 1 * SW + hi],
                            start=False, stop=True,
                        )
                        evac(yat[:, ob * SW + lo : ob * SW + hi], ps[:, :n])

                    # block1: 128x1o, 3 m-components
                    for m in range(3):
                        ps = pst(f"ps_b1_{m}")
                        t = 2 + m
                        nc.tensor.matmul(
                            ps[:, :n], w1t[:], xat[:, t * SW + lo : t * SW + hi],
                            start=True, stop=True,
                        )
                        evac(yat[:, t * SW + lo : t * SW + hi], ps[:, :n])

                    # block2: m-pairs via block-diag W2 (full PE width)
                    for g in range(2):
                        ps = pst(f"ps_b2_{g}")
                        t = 5 + g
                        nc.tensor.matmul(
                            ps[:, :n], w2dt[:], xat[:, t * SW + lo : t * SW + hi],
                            start=True, stop=True,
                        )
                        evac(yat[:, t * SW + lo : t * SW + hi], ps[:, :n])

                    # block2 m=4: plain 64-wide matmul
                    ps = pst("ps_b2_4")
                    nc.tensor.matmul(
                        ps[0:64, :n], w2st[:], xbt[:, lo:hi],
                        start=True, stop=True,
                    )
                    evac(ybt[:, lo:hi], ps[0:64, :n])

                # stores on the ACT ring
                nc.scalar.dma_start(ya[w], yat[:])
                nc.scalar.dma_start(ybd[w], ybt[:])

    nc.compile()
    _BUILD_CACHE[key] = nc
    return nc


TRACE = False
LAST_RESULT = None


def kernel(x, W0, W1, W2):
    from concourse import bass_utils

    nc = _build_program()

    npdt = np.float16 if DT_IO == "float16" else None
    if npdt is None:
        import ml_dtypes
        npdt = ml_dtypes.bfloat16

    w0s = (np.asarray(W0, np.float32) / np.sqrt(256.0)).astype(npdt)
    w1s = (np.asarray(W1, np.float32) / np.sqrt(128.0)).astype(npdt)
    w2 = (np.asarray(W2, np.float32) / np.sqrt(64.0)).astype(npdt)
    wp = np.zeros((128, 832), dtype=npdt)
    wp[:, 0:256] = w0s[0:128]
    wp[:, 256:512] = w0s[128:256]
    wp[:, 512:640] = w1s
    wp[0:64, 640:704] = w2
    wp[64:128, 704:768] = w2
    wp[0:64, 768:832] = w2

    xh = np.asarray(x)[:, _PERM].astype(npdt)
    A = xh.reshape(NCORES, NW, SW, D)
    xa_all = np.ascontiguousarray(
        A[:, :, :, :896].reshape(NCORES, NW, SW, 7, 128).transpose(0, 1, 4, 3, 2)
    ).reshape(NCORES, NW, 128, 7 * SW)
    xb_all = np.ascontiguousarray(A[:, :, :, 896:].transpose(0, 1, 3, 2))

    in_maps = []
    for c in range(NCORES):
        in_maps.append({"xa": xa_all[c], "xb": xb_all[c], "wp": wp})

    res = bass_utils.run_bass_kernel_spmd(
        nc, in_maps, core_ids=list(range(NCORES)), trace=TRACE
    )
    global LAST_RESULT
    LAST_RESULT = res

    out = np.empty((N_TOTAL, D), dtype=np.float32)
    Yp = np.empty((NCORES, NW, SW, D), dtype=np.float32)
    for c in range(NCORES):
        yac = res.results[c]["ya"]    # [NW, 128, 7*SW]
        ybc = res.results[c]["yb"]    # [NW, 64, SW]
        Yp[c, :, :, :896] = (
            yac.reshape(NW, 128, 7, SW).transpose(0, 3, 2, 1).reshape(NW, SW, 896)
        )
        Yp[c, :, :, 896:] = ybc.transpose(0, 2, 1)
    out[:, _PERM] = Yp.reshape(N_TOTAL, D)
    return out


# revision 13
# speedup vs baseline: 1.1814x; 1.0994x over previous
"""IrrepsLinear Trainium2 kernel: y = per-irrep-block x @ W / sqrt(mul).

Irreps layout: 256x0e + 128x1o + 64x2e -> blocks of width 256*1, 128*3, 64*5.
Data-parallel over 8 NeuronCores: each core gets 12500 nodes.

Strategy (v2):
  - fp16 DRAM IO halves HBM traffic (the roofline-binding resource);
    matmuls run fp16 x fp16 -> fp32 PSUM, evac casts back to fp16.
  - Host pre-permutes features so each 128-row K-group of the matmuls is a
    contiguous partition tile, and pre-packs each node-window into a fully
    contiguous DRAM region: 2 loads + 2 stores per window, no strided APs.
  - Block2's five 64-wide m-components: pairs (m0,m1), (m2,m3) are computed
    with a single 128x128 block-diagonal W2 stationary (full PE width);
    m4 runs as a plain 64-wide matmul.
  - 8 PSUM banks (one per logical output tile per 512-slice), rotating;
    evac alternates DVE / ACT engines; loads on SP HWDGE ring, stores on
    ACT HWDGE ring.
"""

import numpy as np

NCORES = 8
N_TOTAL = 100000
NSH = N_TOTAL // NCORES   # 12500 nodes per core
NW = 5                    # windows per core
SW = NSH // NW            # 2500 node columns per window
D = 960
MMW = 512                 # matmul slice width (= one fp32 PSUM bank)

DT_IO = "float16"         # DRAM/SBUF dtype ("float16" | "bfloat16")
_BUILD_CACHE = {}

# feature permutation: tile-row order -> original feature index
# t0,t1: block0 (256 features); t2..t4: block1 m=0..2 (128 each);
# t5: block2 (m0|m1), t6: (m2|m3)  [paired via block-diag W2]; xb: m4 (64).
def _perm():
    p = list(range(256))
    for m in range(3):
        p += [256 + 3 * i + m for i in range(128)]
    for m in range(5):
        p += [640 + 5 * i + m for i in range(64)]
    return np.asarray(p, dtype=np.int64)

_PERM = _perm()


def _build_program():
    import concourse.bass as bass  # noqa: F401
    import concourse.bacc as bacc
    import concourse.mybir as mybir
    import concourse.tile as tile

    key = (DT_IO, SW, MMW, NW)
    if key in _BUILD_CACHE:
        return _BUILD_CACHE[key]

    dt = getattr(mybir.dt, DT_IO)
    f32 = mybir.dt.float32

    nc = bacc.Bacc(
        "TRN2", target_bir_lowering=False, debug=False, enable_asserts=False
    )
    xa = nc.dram_tensor("xa", [NW, 128, 7 * SW], dt, kind="ExternalInput").ap()
    xb = nc.dram_tensor("xb", [NW, 64, SW], dt, kind="ExternalInput").ap()
    w0 = nc.dram_tensor("w0", [256, 256], dt, kind="ExternalInput").ap()
    w1 = nc.dram_tensor("w1", [128, 128], dt, kind="ExternalInput").ap()
    w2d = nc.dram_tensor("w2d", [128, 128], dt, kind="ExternalInput").ap()
    w2s = nc.dram_tensor("w2s", [64, 64], dt, kind="ExternalInput").ap()
    ya = nc.dram_tensor("ya", [NW, 128, 7 * SW], dt, kind="ExternalOutput").ap()
    yb = nc.dram_tensor("yb", [NW, 64, SW], dt, kind="ExternalOutput").ap()

    with tile.TileContext(nc) as tc:
        with (
            tc.tile_pool(name="const", bufs=1) as cpool,
            tc.tile_pool(name="xin", bufs=2) as xpool,
            tc.tile_pool(name="yst", bufs=2) as ypool,
            tc.tile_pool(name="ps", bufs=8, space="PSUM") as pspool,
        ):
            # --- stationary weights, loaded once ---
            w0t0 = cpool.tile([128, 256], dt, name="w0t0", tag="w0t0")
            nc.sync.dma_start(w0t0[:], w0[0:128, :])
            w0t1 = cpool.tile([128, 256], dt, name="w0t1", tag="w0t1")
            nc.sync.dma_start(w0t1[:], w0[128:256, :])
            w1t = cpool.tile([128, 128], dt, name="w1t", tag="w1t")
            nc.sync.dma_start(w1t[:], w1[:, :])
            w2dt = cpool.tile([128, 128], dt, name="w2dt", tag="w2dt")
            nc.sync.dma_start(w2dt[:], w2d[:, :])
            w2st = cpool.tile([64, 64], dt, name="w2st", tag="w2st")
            nc.sync.dma_start(w2st[:], w2s[:, :])

            n_evac = 0

            def evac(dst, src):
                nonlocal n_evac
                n_evac += 1
                if n_evac % 2:
                    nc.vector.tensor_copy(dst, src)
                else:
                    nc.scalar.copy(dst, src)

            slices = [
                (i * MMW, min((i + 1) * MMW, SW))
                for i in range((SW + MMW - 1) // MMW)
            ]

            for w in range(NW):
                xat = xpool.tile([128, 7 * SW], dt, name=f"xa{w}", tag="xa")
                nc.sync.dma_start(xat[:], xa[w])
                xbt = xpool.tile([64, SW], dt, name=f"xb{w}", tag="xb")
                nc.sync.dma_start(xbt[:], xb[w])
                yat = ypool.tile([128, 7 * SW], dt, name=f"ya{w}", tag="ya")
                ybt = ypool.tile([64, SW], dt, name=f"yb{w}", tag="yb")

                for lo, hi in slices:
                    n = hi - lo

                    def pst(nm):
                        return pspool.tile(
                            [128, MMW], f32, name=f"{nm}_{w}_{lo}", tag="ps"
                        )

                    # block0: 256x0e (K=256 via 2 accum steps, M=256 via 2 obs)
                    for ob in range(2):
                        ps = pst(f"ps_b0_{ob}")
                        oc = slice(128 * ob, 128 * (ob + 1))
                        nc.tensor.matmul(
                            ps[:, :n], w0t0[:, oc], xat[:, 0 * SW + lo : 0 * SW + hi],
                            start=True, stop=False,
                        )
                        nc.tensor.matmul(
                            ps[:, :n], w0t1[:, oc], xat[:, 1 * SW + lo : 1 * SW + hi],
                            start=False, stop=True,
                        )
                        evac(yat[:, ob * SW + lo : ob * SW + hi], ps[:, :n])

                    # block1: 128x1o, 3 m-components, shared stationary
                    for m in range(3):
                        ps = pst(f"ps_b1_{m}")
                        t = 2 + m
                        nc.tensor.matmul(
                            ps[:, :n], w1t[:], xat[:, t * SW + lo : t * SW + hi],
                            start=True, stop=True,
                        )
                        evac(yat[:, t * SW + lo : t * SW + hi], ps[:, :n])

                    # block2: 64x2e, m-pairs via block-diag W2 (full PE width)
                    for g in range(2):
                        ps = pst(f"ps_b2_{g}")
                        t = 5 + g
                        nc.tensor.matmul(
                            ps[:, :n], w2dt[:], xat[:, t * SW + lo : t * SW + hi],
                            start=True, stop=True,
                        )
                        evac(yat[:, t * SW + lo : t * SW + hi], ps[:, :n])

                    # block2 m=4: plain 64-wide matmul
                    ps = pst("ps_b2_4")
                    nc.tensor.matmul(
                        ps[0:64, :n], w2st[:], xbt[:, lo:hi],
                        start=True, stop=True,
                    )
                    evac(ybt[:, lo:hi], ps[0:64, :n])

                # stores on the ACT HWDGE ring (loads use the SP ring)
                nc.scalar.dma_start(ya[w], yat[:])
                nc.scalar.dma_start(yb[w], ybt[:])

    nc.compile()
    _BUILD_CACHE[key] = nc
    return nc


TRACE = False
LAST_RESULT = None


def kernel(x, W0, W1, W2):
    from concourse import bass_utils

    nc = _build_program()

    npdt = np.float16 if DT_IO == "float16" else None
    if npdt is None:
        import ml_dtypes
        npdt = ml_dtypes.bfloat16

    w0s = (np.asarray(W0, np.float32) / np.sqrt(256.0)).astype(npdt)
    w1s = (np.asarray(W1, np.float32) / np.sqrt(128.0)).astype(npdt)
    w2 = (np.asarray(W2, np.float32) / np.sqrt(64.0)).astype(npdt)
    w2d = np.zeros((128, 128), dtype=npdt)
    w2d[0:64, 0:64] = w2
    w2d[64:128, 64:128] = w2

    # pack x: [8 cores][NW][SW nodes][960 feats] with features permuted so
    # each 128-row K-group is one contiguous partition tile.
    xh = np.asarray(x)[:, _PERM].astype(npdt)
    A = xh.reshape(NCORES, NW, SW, D)
    # xa[c, w, p, t*SW+n] = A[c, w, n, 128*t+p]   (t < 7)
    xa_all = np.ascontiguousarray(
        A[:, :, :, :896].reshape(NCORES, NW, SW, 7, 128).transpose(0, 1, 4, 3, 2)
    ).reshape(NCORES, NW, 128, 7 * SW)
    # xb[c, w, p, n] = A[c, w, n, 896+p]
    xb_all = np.ascontiguousarray(
        A[:, :, :, 896:].transpose(0, 1, 3, 2)
    )

    in_maps = []
    for c in range(NCORES):
        in_maps.append({
            "xa": xa_all[c], "xb": xb_all[c],
            "w0": w0s, "w1": w1s, "w2d": w2d, "w2s": w2,
        })

    res = bass_utils.run_bass_kernel_spmd(
        nc, in_maps, core_ids=list(range(NCORES)), trace=TRACE
    )
    global LAST_RESULT
    LAST_RESULT = res

    out = np.empty((N_TOTAL, D), dtype=np.float32)
    Yp = np.empty((NCORES, NW, SW, D), dtype=np.float32)
    for c in range(NCORES):
        ya = res.results[c]["ya"]    # [NW, 128, 7*SW]
        yb = res.results[c]["yb"]    # [NW, 64, SW]
        Yp[c, :, :, :896] = (
            ya.reshape(NW, 128, 7, SW).transpose(0, 3, 2, 1).reshape(NW, SW, 896)
        )
        Yp[c, :, :, 896:] = yb.transpose(0, 2, 1)
    out[:, _PERM] = Yp.reshape(N_TOTAL, D)
    return out
